# revision 1
# baseline (speedup 1.0000x reference)
"""Trainium2 Bass kernel for nn_EquiConv2d (equirectangular deformable conv).

Key structural facts exploited (derived from the reference geometry):
  * off_y is exactly longitude-invariant, so each (tap k, row h) samples two
    fixed input rows (iy0, iy0+1) with a constant y-fraction.
  * off_x is longitude-invariant up to the 2*pi wrap: sampling along a row is
    a CIRCULAR shift by a constant s0(k,h) plus a constant x-fraction.
  * Hence the whole deformable conv is 18 matmuls per output row
    ([128=(c x row-pair) contraction, 512 free]) reading circularly
    duplicated row-pair tiles at per-(k,h) column offsets, with the bilinear
    corner weights folded into the stationary (weight) operand.
  * The per-(k,h) column offsets are per-core data: loaded into PE registers
    from an int32 table and applied as dynamic AP slices, so all 8 cores run
    ONE SPMD program.
  * Two fp32 oddities handled exactly: tap (k=7,h=255) is identically zero
    (py==256.0 -> all corners invalid) and tap (k=1,h=1) samples near the
    antipode with fp32-noise-scattered positions -> handled by 3 extra
    matmul slots with per-column coefficient vectors (data-driven, active
    only on the cores owning global row 1).

Sharding: 8 cores = 2 batches x 4 bands of 64 output rows.
"""

import math

import numpy as np

# ----------------------------------------------------------------------------
# problem constants
B, C, H, W = 2, 64, 256, 512
O, KH, KW = 64, 3, 3
K = KH * KW
NCORES = 8
NROW = 64            # output rows per core
NSLOT = 2 * K        # standard matmul slots per row
NSPEC = 3            # special (antipode) slots, accumulated into local row 1
RING = 16            # staged row-pair ring slots
PF = 3               # staging prefetch lead (rows)
SLOTW = 2048         # F(1024) + G(1024) columns per ring slot
GOFF = 1024
SKIP_TOL = 1e-4       # drop matmul slots with |weight| below this

_CACHE = {}


# ----------------------------------------------------------------------------
# host-side geometry tables (must replicate reference fp32 semantics exactly)

def _compute_offsets_jax():
    """Bit-exact replica of reference.equi_offsets on jax CPU."""
    import jax
    import jax.numpy as jnp
    cpu = jax.devices("cpu")[0]
    with jax.default_device(cpu):
        dtype = jnp.float32
        pano_H, pano_W, kH, kW = H, W, KH, KW
        Kk = kH * kW
        u = jnp.arange(pano_W, dtype=dtype)
        v = jnp.arange(pano_H, dtype=dtype)
        phi = (u - pano_W / 2.0) / pano_W * (2.0 * math.pi)
        theta = -(v - pano_H / 2.0) / pano_H * math.pi
        cp, sp = jnp.cos(phi), jnp.sin(phi)
        z, one = jnp.zeros_like(cp), jnp.ones_like(cp)
        Ry = jnp.stack([jnp.stack([cp, z, sp], -1),
                        jnp.stack([z, one, z], -1),
                        jnp.stack([-sp, z, cp], -1)], -2)
        ct, st = jnp.cos(theta), jnp.sin(theta)
        zh, oh = jnp.zeros_like(ct), jnp.ones_like(ct)
        Rx = jnp.stack([jnp.stack([oh, zh, zh], -1),
                        jnp.stack([zh, ct, -st], -1),
                        jnp.stack([zh, st, ct], -1)], -2)
        ROT = jnp.einsum('wij,hjk->hwik', Ry, Rx)
        fov_w = kW * (2.0 * math.pi / pano_W)
        focal = (kW / 2.0) / math.tan(fov_w / 2.0)
        hg = (jnp.arange(kH, dtype=dtype)[:, None] + 0.5 - kH / 2.0)
        wg = (jnp.arange(kW, dtype=dtype)[None, :] + 0.5 - kW / 2.0)
        hg = jnp.broadcast_to(hg, (kH, kW)).reshape(Kk)
        wg = jnp.broadcast_to(wg, (kH, kW)).reshape(Kk)
        rays0 = jnp.stack([wg / focal, hg / focal, jnp.ones(Kk, dtype)], 0)
        rays0 = rays0 / jnp.linalg.norm(rays0, axis=0, keepdims=True)
        rays = jnp.einsum('hwik,kn->hwin', ROT, rays0)
        phi2 = jnp.arctan2(rays[..., 0, :], rays[..., 2, :])
        th2 = jnp.arcsin(jnp.clip(rays[..., 1, :], -1.0, 1.0))
        x = pano_W / (2.0 * math.pi) * phi2 + pano_W / 2.0
        y = pano_H / math.pi * th2 + pano_H / 2.0
        off_x = x - (wg[None, None, :] + u[None, :, None])
        off_y = y - (hg[None, None, :] + v[:, None, None])
        return (np.asarray(jnp.transpose(off_y, (2, 0, 1))),
                np.asarray(jnp.transpose(off_x, (2, 0, 1))))


def _build_tap_tables():
    off_y, off_x = _compute_offsets_jax()
    ky = np.repeat(np.arange(KH), KW).astype(np.float32)
    kx = np.tile(np.arange(KW), KH).astype(np.float32)
    base_x = (np.arange(W, dtype=np.float32) - np.float32(1))
    base_y = (np.arange(H, dtype=np.float32) - np.float32(1))
    px = (base_x[None, None, :] + kx[:, None, None] + off_x).astype(np.float32)
    py = (base_y[None, :, None] + ky[:, None, None] + off_y).astype(np.float32)
    pyc = py[:, :, 0]
    assert np.all(py == pyc[:, :, None]), "off_y not longitude-invariant"

    iy0 = np.floor(pyc).astype(np.int64)
    wy1 = (pyc - np.floor(pyc)).astype(np.float64)
    v0 = (iy0 >= 0) & (iy0 < H)
    v1 = (iy0 + 1 >= 0) & (iy0 + 1 < H)
    cy0 = np.where(v0, 1.0 - wy1, 0.0)
    cy1 = np.where(v1, wy1, 0.0)

    Draw = np.mod((px.astype(np.float64) - np.arange(W)[None, None, :]), 512.0)
    ang = Draw / 512.0 * 2 * np.pi
    mean = np.mod(np.angle(np.exp(1j * ang).mean(axis=2)) / (2 * np.pi) * 512.0,
                  512.0)
    resid = np.mod(Draw - mean[:, :, None] + 256.0, 512.0) - 256.0
    D = mean + np.median(resid, axis=2)
    s0 = np.mod(np.floor(D), 512).astype(np.int64)
    frac = D - np.floor(D)

    special = np.zeros((K, H), dtype=bool)
    special[1, 1] = True
    dead = (cy0 == 0.0) & (cy1 == 0.0)

    Ddev = np.abs(np.mod(Draw - D[:, :, None] + 256.0, 512.0) - 256.0)
    dev = Ddev.max(axis=2)
    bad = (dev > 5e-4) & ~special & ~dead
    assert not bad.any(), f"unrepresentable taps: {np.argwhere(bad)}"

    def ref_coefs(p):
        x0 = math.floor(p)
        fr = p - x0
        out = {}
        for ix, wt in ((x0, 1.0 - fr), (x0 + 1, fr)):
            if 0 <= ix < W and wt != 0.0:
                out[ix] = out.get(ix, 0.0) + wt
        return out

    # seam variant selection: decided by the exact fp32 px at the wrap column
    slot0_useG = np.zeros((K, H), dtype=bool)
    slot1_useF = np.zeros((K, H), dtype=bool)
    for k in range(K):
        for h in range(H):
            if special[k, h] or dead[k, h]:
                continue
            s = int(s0[k, h]); fr = frac[k, h]
            if s >= 1:
                w0 = (512 - s) % 512
                rc = ref_coefs(float(px[k, h, w0]))
                slot0_useG[k, h] = (abs(rc.get(0, 0.0))
                                    < abs(rc.get(0, 0.0) - (1 - fr)))
            w1 = (511 - s) % 512
            rc = ref_coefs(float(px[k, h, w1]))
            slot1_useF[k, h] = (abs(rc.get(0, 0.0) - fr)
                                < abs(rc.get(0, 0.0)))

    # special tap (1,1): per-column coefficients on F offsets 255..257
    pxs = px[1, 1, :].astype(np.float64)
    Gam = np.zeros((3, W), dtype=np.float64)
    for w in range(W):
        p = pxs[w]
        x0 = math.floor(p)
        fr = p - x0
        for ix, wt in ((x0, 1.0 - fr), (x0 + 1, fr)):
            if 0 <= ix < W and wt != 0.0:
                found = False
                for jj in range(3):
                    if (255 + jj + w) % 512 == ix % 512:
                        Gam[jj, w] += wt
                        found = True
                        break
                assert found, (w, p, ix)

    return dict(iy0=iy0, cy0=cy0, cy1=cy1, s0=s0, frac=frac,
                slot0_useG=slot0_useG, slot1_useF=slot1_useF,
                special=special, dead=dead, Gam=Gam)


# ----------------------------------------------------------------------------
# uniform SPMD schedule

def _build_schedule(tt):
    blocks = []
    for blk in range(4):
        h0 = blk * NROW
        ev_of, events, first_use = {}, [], []
        need = np.zeros((NROW, K), np.int64)
        for lh in range(NROW):
            for k in range(K):
                r = int(np.clip(tt['iy0'][k, h0 + lh], 0, 255))
                if r not in ev_of:
                    ev_of[r] = len(events)
                    events.append(r)
                    first_use.append(lh)
                need[lh, k] = ev_of[r]
        blocks.append(dict(events=events, first_use=first_use, need=need))

    E = max(len(b['events']) for b in blocks)
    for b in blocks:
        while len(b['events']) < E:
            b['events'].append(b['events'][-1])

    # uniform staged-count before row lh:  tgt(lh) = U[min(lh+PF, NROW-1)]
    U = np.zeros(NROW, np.int64)
    for lh in range(NROW):
        U[lh] = max(int(np.searchsorted(np.asarray(b['first_use']), lh, 'right'))
                    for b in blocks)
    tgt = np.array([U[min(lh + PF, NROW - 1)] for lh in range(NROW)])

    # ring-overwrite feasibility
    ls = np.full(E, NROW, np.int64)
    for e in range(E):
        hit = np.where(tgt > e)[0]
        if len(hit):
            ls[e] = hit[0]
    for b in blocks:
        lastuse = {}
        for lh in range(NROW):
            for k in range(K):
                lastuse[int(b['need'][lh, k])] = lh
        for e in range(RING, E):
            prev = e - RING
            if prev in lastuse:
                assert lastuse[prev] < ls[e], \
                    f"RING={RING} too small: ev{e} overwrites ev{prev} " \
                    f"(lastuse {lastuse[prev]}, staged before row {ls[e]})"
    espc = int(blocks[0]['need'][1, 1])
    return blocks, E, tgt, espc


def _build_scale_tables(tt):
    """Per-block fp64 scale vectors [NROW, NSLOT, 128] (geometry only)."""
    scs = []
    for blk in range(4):
        h0 = blk * NROW
        sc = np.zeros((NROW, NSLOT, 128), np.float64)
        for lh in range(NROW):
            h = h0 + lh
            for k in range(K):
                if tt['dead'][k, h] or tt['special'][k, h]:
                    continue
                fr = tt['frac'][k, h]
                c0, c1 = tt['cy0'][k, h], tt['cy1'][k, h]
                sc[lh, 2 * k, :64] = c0 * (1 - fr)
                sc[lh, 2 * k, 64:] = c1 * (1 - fr)
                sc[lh, 2 * k + 1, :64] = c0 * fr
                sc[lh, 2 * k + 1, 64:] = c1 * fr
        scs.append(sc)
    return scs


# ----------------------------------------------------------------------------
# device program

def _emit_section(tc, aps, tiles, tt, blkinfo, j):
    """Emit one per-band section (all-static APs)."""
    import concourse.mybir as mybir
    nc = tc.nc
    f16 = mybir.dt.float16
    f32 = mybir.dt.float32
    buf, coeft, biast, ltst = tiles
    xb, outd, lt = aps['xb'], aps['out'], aps['lt']
    need = blkinfo['need']
    first_use = blkinfo['first_use']
    E_j = len(first_use)

    cum = [int(np.searchsorted(np.asarray(first_use), lh, 'right'))
           for lh in range(NROW)]
    tgt = [cum[min(lh + PF, NROW - 1)] for lh in range(NROW)]

    ls = [NROW] * E_j
    for e in range(E_j):
        for lh in range(NROW):
            if tgt[lh] > e:
                ls[e] = lh
                break
    lastuse = {}
    for lh in range(NROW):
        for k in range(K):
            lastuse[int(need[lh, k])] = lh
    for e in range(RING, E_j):
        if e - RING in lastuse:
            assert lastuse[e - RING] < ls[e], (j, e)

    def stage(e):
        base = (e % RING) * SLOTW
        src = xb[e].rearrange("p c w -> (p c) w")
        nc.sync.dma_start(buf[:, base:base + W], src)
        nc.sync.dma_start(buf[:, base + W:base + 2 * W], src)
        nc.vector.tensor_copy(buf[:, base + GOFF:base + GOFF + W],
                              buf[:, base + 1:base + 1 + W])
        nc.scalar.copy(buf[:, base + GOFF + W:base + GOFF + 2 * W],
                       buf[:, base + 1:base + 1 + W])
        nc.gpsimd.memset(buf[:, base + GOFF + 511:base + GOFF + 512], 0.0)

    psp, ltp, zp, outp = tiles_pools[0]

    staged = 0
    for lh in range(NROW):
        while staged < tgt[lh]:
            stage(staged)
            staged += 1
        h = j * NROW + lh
        ltt = ltp.tile([128, NSLOT * O], f16, tag="ltt")
        nc.sync.dma_start(ltt, lt[lh])
        ps = psp.tile([O, W], f32, tag="ps")
        # collect (slot, rhs-offset) slots, statically skipping near-zero
        # weights: slot1 weight=frac, slot0 weight=1-frac; drop < SKIP_TOL
        emits = []
        for k in range(K):
            base = int(need[lh, k] % RING) * SLOTW
            s = int(tt['s0'][k, h])
            if tt['dead'][k, h] or tt['special'][k, h]:
                continue
            fr = float(tt['frac'][k, h])
            if tt['slot0_useG'][k, h] and s >= 1:
                v0 = base + GOFF + s - 1
            else:
                v0 = base + s
            v1 = base + s + 1 if tt['slot1_useF'][k, h] \
                else base + GOFF + s
            if 1.0 - fr >= SKIP_TOL:
                emits.append((2 * k, v0))
            if fr >= SKIP_TOL:
                emits.append((2 * k + 1, v1))
        nmm = len(emits) + (NSPEC if (j == 0 and lh == 1) else 0)
        mi = 0
        for sl, v in emits:
            nc.tensor.matmul(ps, ltt[:, sl * O:(sl + 1) * O],
                             buf[:, v:v + W],
                             start=(mi == 0), stop=(mi == nmm - 1))
            mi += 1
        if j == 0 and lh == 1:
            sbase = int(need[1, 1] % RING) * SLOTW
            for jj in range(NSPEC):
                zt = zp.tile([128, W], f16, tag="spz")
                nc.vector.tensor_mul(
                    zt, buf[:, sbase + 255 + jj:sbase + 255 + jj + W],
                    coeft[:, jj * W:(jj + 1) * W])
                nc.tensor.matmul(ps, ltst[:, jj * O:(jj + 1) * O], zt,
                                 start=False, stop=(mi == nmm - 1))
                mi += 1
        ot = outp.tile([O, W], f32, tag="out")
        nc.scalar.activation(ot, ps,
                             mybir.ActivationFunctionType.Identity,
                             bias=biast, scale=1.0)
        nc.sync.dma_start(outd[lh], ot)


tiles_pools = [None]


def _emit_kernel(tc, aps, tt, blocks):
    import concourse.mybir as mybir
    nc = tc.nc
    f16 = mybir.dt.float16
    f32 = mybir.dt.float32

    with tc.tile_pool(name="bigp", bufs=1) as bigp, \
         tc.tile_pool(name="ltp", bufs=3) as ltp, \
         tc.tile_pool(name="zp", bufs=3) as zp, \
         tc.tile_pool(name="psp", bufs=6, space="PSUM") as psp, \
         tc.tile_pool(name="outp", bufs=3) as outp:

        buf = bigp.tile([128, RING * SLOTW], f16)
        coeft = bigp.tile([128, NSPEC * W], f16)
        biast = bigp.tile([O, 1], f32)
        ltst = bigp.tile([128, NSPEC * O], f16)

        blkv = nc.values_load(aps['blkid'][0:1, 0:1],
                              min_val=0, max_val=3,
                              skip_runtime_bounds_check=True)

        nc.sync.dma_start(coeft, aps['coefr'])
        nc.sync.dma_start(biast, aps['biasd'])
        nc.sync.dma_start(ltst, aps['lts'])

        tiles = (buf, coeft, biast, ltst)
        tiles_pools[0] = (psp, ltp, zp, outp)
        for j in range(4):
            with tc.If(blkv == j):
                _emit_section(tc, aps, tiles, tt, blocks[j], j)


def _get_compiled():
    """Build tables, schedule, and the Bass program once."""
    if 'prog' in _CACHE:
        return _CACHE['prog']
    import concourse.mybir as mybir
    import concourse.tile as tile
    from concourse import bacc

    tt = _build_tap_tables()
    blocks, E, _tgt, _espc = _build_schedule(tt)
    scs = _build_scale_tables(tt)

    f16 = mybir.dt.float16
    f32 = mybir.dt.float32
    nc = bacc.Bacc("TRN2", target_bir_lowering=False, debug=False,
                   num_devices=NCORES)
    aps = {
        'xb': nc.dram_tensor("xb", [E, 2, C, W], f16,
                             kind="ExternalInput").ap(),
        'lt': nc.dram_tensor("lt", [NROW, 128, NSLOT * O], f16,
                             kind="ExternalInput").ap(),
        'lts': nc.dram_tensor("lts", [128, NSPEC * O], f16,
                              kind="ExternalInput").ap(),
        'blkid': nc.dram_tensor("blkid", [1, 1], mybir.dt.int32,
                                kind="ExternalInput").ap(),
        'coefr': nc.dram_tensor("coefr", [128, NSPEC * W], f16,
                                kind="ExternalInput").ap(),
        'biasd': nc.dram_tensor("biasd", [O, 1], f32,
                                kind="ExternalInput").ap(),
        'out': nc.dram_tensor("out", [NROW, O, W], f32,
                              kind="ExternalOutput").ap(),
    }
    with tile.TileContext(nc) as tc:
        _emit_kernel(tc, aps, tt, blocks)
    nc.finalize()

    _CACHE['prog'] = (nc, tt, blocks, E, scs)
    return _CACHE['prog']


def _core_inputs(x, weight, bias, tt, blocks, E, scs):
    """Assemble per-core in_maps. Core c = batch (c // 4), band (c % 4)."""
    w3 = weight.reshape(O, C, K).astype(np.float64)
    # W2d[p, k, o]: channel-duplicated weights on the contraction axis
    w2d = np.empty((128, K, O), np.float64)
    w2d[:C] = w3.transpose(1, 2, 0)
    w2d[C:] = w3.transpose(1, 2, 0)
    # slot-expanded: [NSLOT, 128, O]
    w2s = np.repeat(w2d.transpose(1, 0, 2), 2, axis=0)
    biasd = np.ascontiguousarray(bias.reshape(O, 1).astype(np.float32))

    lts_on = np.zeros((128, NSPEC * O), np.float16)
    for jj in range(NSPEC):
        lts_on[:C, jj * O:(jj + 1) * O] = w2d[:C, 1, :].astype(np.float16)
    lts_off = np.zeros((128, NSPEC * O), np.float16)

    Gam = tt['Gam'].astype(np.float16)
    coef_on = np.ascontiguousarray(
        np.broadcast_to(Gam[:, None, :], (NSPEC, 128, W))
        .transpose(1, 0, 2).reshape(128, NSPEC * W))
    coef_off = np.zeros((128, NSPEC * W), np.float16)

    lt_blk = []
    for blk in range(4):
        # [l, s, p] x [s, p, o] -> [l, p, s, o]
        ltv = np.einsum('lsp,spo->lpso', scs[blk], w2s)
        lt_blk.append(np.ascontiguousarray(
            ltv.reshape(NROW, 128, NSLOT * O)).astype(np.float16))

    in_maps = []
    for cid in range(NCORES):
        b, blk = cid // 4, cid % 4
        xz = np.concatenate([x[b], np.zeros((C, 1, W), x.dtype)], axis=1)
        xz = xz.astype(np.float16)
        rows = np.asarray(blocks[blk]['events'], np.int64)
        pair_idx = np.stack([rows, rows + 1], axis=1)       # [E, 2]
        xbv = xz[:, pair_idx, :]                            # [C, E, 2, W]
        xbv = np.ascontiguousarray(xbv.transpose(1, 2, 0, 3))  # [E,2,C,W]
        in_maps.append({
            'xb': xbv,
            'lt': lt_blk[blk],
            'lts': lts_on if blk == 0 else lts_off,
            'blkid': np.array([[blk]], np.int32),
            'coefr': coef_on if blk == 0 else coef_off,
            'biasd': biasd,
        })
    return in_maps


def kernel(x, weight, bias):
    from concourse.bass_utils import run_bass_kernel_spmd
    x = np.asarray(x, dtype=np.float32)
    weight = np.asarray(weight, dtype=np.float32)
    bias = np.asarray(bias, dtype=np.float32)

    nc, tt, blocks, E, scs = _get_compiled()
    in_maps = _core_inputs(x, weight, bias, tt, blocks, E, scs)
    res = run_bass_kernel_spmd(nc, in_maps, core_ids=list(range(NCORES)))

    out = np.empty((B, O, H, W), np.float32)
    for cid in range(NCORES):
        b, blk = cid // 4, cid % 4
        oc = res.results[cid]['out']                        # [NROW, O, W]
        out[b, :, blk * NROW:(blk + 1) * NROW, :] = oc.transpose(1, 0, 2)
    return out



# revision 4
# speedup vs baseline: 1.4580x; 1.4580x over previous
"""Trainium2 Bass kernel for nn_EquiConv2d (equirectangular deformable conv).

Key structural facts exploited (derived from the reference geometry):
  * off_y is exactly longitude-invariant, so each (tap k, row h) samples two
    fixed input rows (iy0, iy0+1) with a constant y-fraction.
  * off_x is longitude-invariant up to the 2*pi wrap: sampling along a row is
    a CIRCULAR shift by a constant s0(k,h) plus a constant x-fraction.
  * Hence the deformable conv is a set of matmul "slots" per output row
    ([128=(c x row-pair) contraction, 512 free]) reading circularly
    duplicated row-pair tiles at per-(k,h) column offsets, with the bilinear
    corner weights folded into the stationary (weight) operand.
  * NEW vs baseline: output rows are processed in PAIRS sharing one PSUM
    bank (top 64 partitions = even row, bottom 64 = odd row).  Slots of the
    two rows that read the SAME moving stream (same input row-pair event,
    same column window, same wrap-seam variant) are MERGED into a single
    matmul with a [128, 128] stationary — the ky-ladder of the equirect
    geometry makes ~30% of all slots mergeable, cutting tensor-engine work
    by the same fraction with bit-identical arithmetic.
  * Two fp32 oddities handled exactly: tap (k=7,h=255) is identically zero
    and tap (k=1,h=1) samples near the antipode with fp32-noise-scattered
    positions -> handled by 3 extra matmul slots with per-column coefficient
    vectors (data-driven, active only on the cores owning global row 1).

Sharding: 8 cores = 2 batches x 4 bands of 32 output-row pairs.
"""

import math

import numpy as np

# ----------------------------------------------------------------------------
# problem constants
B, C, H, W = 2, 64, 256, 512
O, KH, KW = 64, 3, 3
K = KH * KW
NCORES = 8
NROW = 64            # output rows per core
NPAIR = NROW // 2    # output row-pairs per core
NSPEC = 3            # special (antipode) slots, accumulated into local row 1
RING = 16            # staged row-pair ring slots
PF = 2               # staging prefetch lead (row-pairs)
LTLEAD = 2           # lt-table DMA prefetch lead (row-pairs)
SLOTW = 2048         # F(1024) + G(1024) columns per ring slot
GOFF = 1024
SKIP_TOL = 1e-4      # drop matmul slots with |scale| below this

_CACHE = {}


# ----------------------------------------------------------------------------
# host-side geometry tables (must replicate reference fp32 semantics exactly)

def _compute_offsets_jax():
    """Bit-exact replica of reference.equi_offsets on jax CPU."""
    import jax
    import jax.numpy as jnp
    cpu = jax.devices("cpu")[0]
    with jax.default_device(cpu):
        dtype = jnp.float32
        pano_H, pano_W, kH, kW = H, W, KH, KW
        Kk = kH * kW
        u = jnp.arange(pano_W, dtype=dtype)
        v = jnp.arange(pano_H, dtype=dtype)
        phi = (u - pano_W / 2.0) / pano_W * (2.0 * math.pi)
        theta = -(v - pano_H / 2.0) / pano_H * math.pi
        cp, sp = jnp.cos(phi), jnp.sin(phi)
        z, one = jnp.zeros_like(cp), jnp.ones_like(cp)
        Ry = jnp.stack([jnp.stack([cp, z, sp], -1),
                        jnp.stack([z, one, z], -1),
                        jnp.stack([-sp, z, cp], -1)], -2)
        ct, st = jnp.cos(theta), jnp.sin(theta)
        zh, oh = jnp.zeros_like(ct), jnp.ones_like(ct)
        Rx = jnp.stack([jnp.stack([oh, zh, zh], -1),
                        jnp.stack([zh, ct, -st], -1),
                        jnp.stack([zh, st, ct], -1)], -2)
        ROT = jnp.einsum('wij,hjk->hwik', Ry, Rx)
        fov_w = kW * (2.0 * math.pi / pano_W)
        focal = (kW / 2.0) / math.tan(fov_w / 2.0)
        hg = (jnp.arange(kH, dtype=dtype)[:, None] + 0.5 - kH / 2.0)
        wg = (jnp.arange(kW, dtype=dtype)[None, :] + 0.5 - kW / 2.0)
        hg = jnp.broadcast_to(hg, (kH, kW)).reshape(Kk)
        wg = jnp.broadcast_to(wg, (kH, kW)).reshape(Kk)
        rays0 = jnp.stack([wg / focal, hg / focal, jnp.ones(Kk, dtype)], 0)
        rays0 = rays0 / jnp.linalg.norm(rays0, axis=0, keepdims=True)
        rays = jnp.einsum('hwik,kn->hwin', ROT, rays0)
        phi2 = jnp.arctan2(rays[..., 0, :], rays[..., 2, :])
        th2 = jnp.arcsin(jnp.clip(rays[..., 1, :], -1.0, 1.0))
        x = pano_W / (2.0 * math.pi) * phi2 + pano_W / 2.0
        y = pano_H / math.pi * th2 + pano_H / 2.0
        off_x = x - (wg[None, None, :] + u[None, :, None])
        off_y = y - (hg[None, None, :] + v[:, None, None])
        return (np.asarray(jnp.transpose(off_y, (2, 0, 1))),
                np.asarray(jnp.transpose(off_x, (2, 0, 1))))


def _build_tap_tables():
    off_y, off_x = _compute_offsets_jax()
    ky = np.repeat(np.arange(KH), KW).astype(np.float32)
    kx = np.tile(np.arange(KW), KH).astype(np.float32)
    base_x = (np.arange(W, dtype=np.float32) - np.float32(1))
    base_y = (np.arange(H, dtype=np.float32) - np.float32(1))
    px = (base_x[None, None, :] + kx[:, None, None] + off_x).astype(np.float32)
    py = (base_y[None, :, None] + ky[:, None, None] + off_y).astype(np.float32)
    pyc = py[:, :, 0]
    assert np.all(py == pyc[:, :, None]), "off_y not longitude-invariant"

    iy0 = np.floor(pyc).astype(np.int64)
    wy1 = (pyc - np.floor(pyc)).astype(np.float64)
    v0 = (iy0 >= 0) & (iy0 < H)
    v1 = (iy0 + 1 >= 0) & (iy0 + 1 < H)
    cy0 = np.where(v0, 1.0 - wy1, 0.0)
    cy1 = np.where(v1, wy1, 0.0)

    Draw = np.mod((px.astype(np.float64) - np.arange(W)[None, None, :]), 512.0)
    ang = Draw / 512.0 * 2 * np.pi
    mean = np.mod(np.angle(np.exp(1j * ang).mean(axis=2)) / (2 * np.pi) * 512.0,
                  512.0)
    resid = np.mod(Draw - mean[:, :, None] + 256.0, 512.0) - 256.0
    D = mean + np.median(resid, axis=2)
    s0 = np.mod(np.floor(D), 512).astype(np.int64)
    frac = D - np.floor(D)

    special = np.zeros((K, H), dtype=bool)
    special[1, 1] = True
    dead = (cy0 == 0.0) & (cy1 == 0.0)

    Ddev = np.abs(np.mod(Draw - D[:, :, None] + 256.0, 512.0) - 256.0)
    dev = Ddev.max(axis=2)
    bad = (dev > 5e-4) & ~special & ~dead
    assert not bad.any(), f"unrepresentable taps: {np.argwhere(bad)}"

    def ref_coefs(p):
        x0 = math.floor(p)
        fr = p - x0
        out = {}
        for ix, wt in ((x0, 1.0 - fr), (x0 + 1, fr)):
            if 0 <= ix < W and wt != 0.0:
                out[ix] = out.get(ix, 0.0) + wt
        return out

    # seam variant selection: decided by the exact fp32 px at the wrap column
    slot0_useG = np.zeros((K, H), dtype=bool)
    slot1_useF = np.zeros((K, H), dtype=bool)
    for k in range(K):
        for h in range(H):
            if special[k, h] or dead[k, h]:
                continue
            s = int(s0[k, h]); fr = frac[k, h]
            if s >= 1:
                w0 = (512 - s) % 512
                rc = ref_coefs(float(px[k, h, w0]))
                slot0_useG[k, h] = (abs(rc.get(0, 0.0))
                                    < abs(rc.get(0, 0.0) - (1 - fr)))
            w1 = (511 - s) % 512
            rc = ref_coefs(float(px[k, h, w1]))
            slot1_useF[k, h] = (abs(rc.get(0, 0.0) - fr)
                                < abs(rc.get(0, 0.0)))

    # special tap (1,1): per-column coefficients on F offsets 255..257
    pxs = px[1, 1, :].astype(np.float64)
    Gam = np.zeros((3, W), dtype=np.float64)
    for w in range(W):
        p = pxs[w]
        x0 = math.floor(p)
        fr = p - x0
        for ix, wt in ((x0, 1.0 - fr), (x0 + 1, fr)):
            if 0 <= ix < W and wt != 0.0:
                found = False
                for jj in range(3):
                    if (255 + jj + w) % 512 == ix % 512:
                        Gam[jj, w] += wt
                        found = True
                        break
                assert found, (w, p, ix)

    return dict(iy0=iy0, cy0=cy0, cy1=cy1, s0=s0, frac=frac,
                slot0_useG=slot0_useG, slot1_useF=slot1_useF,
                special=special, dead=dead, Gam=Gam)


# ----------------------------------------------------------------------------
# slot -> (event row, window, variant) keys + merged emit schedule

def _row_slots(tt, h):
    """Slots of output row h: list of (key, k, half_scales).
    key = (input_row, window_col, zeroed_variant);
    scales = (coef_row0, coef_row1) to fold into the stationary halves."""
    out = []
    for k in range(K):
        if tt['dead'][k, h] or tt['special'][k, h]:
            continue
        s = int(tt['s0'][k, h]); fr = float(tt['frac'][k, h])
        iy = int(np.clip(tt['iy0'][k, h], 0, 255))
        c0 = float(tt['cy0'][k, h]); c1 = float(tt['cy1'][k, h])
        cmax = max(abs(c0), abs(c1))
        if (1.0 - fr) * cmax >= SKIP_TOL:
            zer = bool(tt['slot0_useG'][k, h]) and s >= 1
            out.append(((iy, s, zer), k, (c0 * (1 - fr), c1 * (1 - fr))))
        if fr * cmax >= SKIP_TOL:
            zer = not bool(tt['slot1_useF'][k, h])
            out.append(((iy, s + 1, zer), k, (c0 * fr, c1 * fr)))
    return out


def _build_schedule(tt):
    """Per band: event list (input row-pairs, in first-use order over pairs),
    staging targets, and per-pair emit lists.

    emit = dict(ev, win, zer, top=[(k,c0,c1)...], bot=[...], ltcol, width)
    ordered merged-first (within groups by event index)."""
    iy_spc = int(np.clip(tt['iy0'][1, 1], 0, 255))
    blocks = []
    for blk in range(4):
        ev_of, events, first_use = {}, [], []
        pairs = []
        for p in range(NPAIR):
            h0 = blk * NROW + 2 * p
            h1 = h0 + 1
            if blk == 0 and p == 0 and iy_spc not in ev_of:
                ev_of[iy_spc] = len(events)
                events.append(iy_spc)
                first_use.append(0)
            keymap = {}
            for (key, k, sc) in _row_slots(tt, h0):
                keymap.setdefault(key, (dict(), dict()))[0].setdefault(
                    k, [0.0, 0.0])
                e = keymap[key][0][k]
                e[0] += sc[0]; e[1] += sc[1]
            for (key, k, sc) in _row_slots(tt, h1):
                keymap.setdefault(key, (dict(), dict()))[1].setdefault(
                    k, [0.0, 0.0])
                e = keymap[key][1][k]
                e[0] += sc[0]; e[1] += sc[1]
            # register events (input rows) in deterministic order
            emits = []
            for key in keymap:
                iy = key[0]
                if iy not in ev_of:
                    ev_of[iy] = len(events)
                    events.append(iy)
                    first_use.append(p)
                top, bot = keymap[key]
                emits.append(dict(ev=ev_of[iy], win=key[1], zer=key[2],
                                  top=top, bot=bot,
                                  merged=bool(top) and bool(bot)))
            # merged first, then solos; within each group by event order
            emits.sort(key=lambda em: (not em['merged'], em['ev'], em['win']))
            # assign lt columns
            col = 0
            for em in emits:
                em['width'] = 128 if em['merged'] else 64
                em['ltcol'] = col
                col += em['width']
            pairs.append(dict(emits=emits, ltw=col))
        blocks.append(dict(events=events, first_use=first_use, pairs=pairs))

    E = max(len(b['events']) for b in blocks)
    LTW = max(pr['ltw'] for b in blocks for pr in b['pairs'])
    for b in blocks:
        while len(b['events']) < E:
            b['events'].append(b['events'][-1])

    # staging target per pair: staged-count needed before pair p is
    # tgt[p] = U[min(p+PF, NPAIR-1)] where U = events first-used by <= p
    for b in blocks:
        fu = np.asarray(b['first_use'])
        Uv = np.array([int(np.searchsorted(fu, p, 'right'))
                       for p in range(NPAIR)])
        b['tgt'] = [int(Uv[min(p + PF, NPAIR - 1)]) for p in range(NPAIR)]
        # ring-overwrite feasibility: event e staged before pair ls[e];
        # it overwrites slot of event e-RING whose last use must precede.
        ls = np.full(E, NPAIR, np.int64)
        tgt = np.asarray(b['tgt'])
        for e in range(E):
            hit = np.where(tgt > e)[0]
            if len(hit):
                ls[e] = hit[0]
        lastuse = {}
        for p in range(NPAIR):
            for em in b['pairs'][p]['emits']:
                lastuse[em['ev']] = p
        for e in range(RING, E):
            if e - RING in lastuse:
                assert lastuse[e - RING] < ls[e], \
                    f"RING={RING} too small: ev{e} overwrites ev{e-RING}"
    return blocks, E, LTW


# ----------------------------------------------------------------------------
# device program

def _emit_pair_section(tc, aps, tiles, blkinfo, j):
    """Emit one per-band section (all-static APs)."""
    import concourse.mybir as mybir
    nc = tc.nc
    f16 = mybir.dt.float16
    f32 = mybir.dt.float32
    buf, coeft, biast, ltst = tiles
    xb, outd, lt = aps['xb'], aps['out'], aps['lt']
    psp, ltp, zp, outp = tiles_pools[0]
    tgt = blkinfo['tgt']
    pairs = blkinfo['pairs']

    def stage(e):
        base = (e % RING) * SLOTW
        src = xb[e].rearrange("p c w -> (p c) w")
        nc.sync.dma_start(buf[:, base:base + W], src)
        nc.sync.dma_start(buf[:, base + W:base + 2 * W], src)
        nc.vector.tensor_copy(buf[:, base + GOFF:base + GOFF + W],
                              buf[:, base + 1:base + 1 + W])
        nc.scalar.copy(buf[:, base + GOFF + W:base + GOFF + 2 * W],
                       buf[:, base + 1:base + 1 + W])
        nc.gpsimd.memset(buf[:, base + GOFF + 511:base + GOFF + 512], 0.0)

    staged = min(RING, len(blkinfo['events']))   # hoisted prologue staging
    ltts = [None] * NPAIR

    def emit_pair(p):
        pr = pairs[p]
        ltt = ltts[p]
        ps = psp.tile([128, W], f32, tag="ps")
        emits = pr['emits']
        nmm = len(emits) + (NSPEC if (j == 0 and p == 0) else 0)
        mi = 0
        started_top = started_bot = False
        for em in emits:
            base = (em['ev'] % RING) * SLOTW
            v = base + em['win'] if not em['zer'] \
                else base + GOFF + em['win'] - 1
            if em['merged']:
                assert not (started_top or started_bot) or \
                    (started_top and started_bot)
                start = not started_top
                started_top = started_bot = True
                out_ap = ps
            elif em['top']:
                start = not started_top
                started_top = True
                out_ap = ps[0:64]
            else:
                start = not started_bot
                started_bot = True
                out_ap = ps[64:128]
            nc.tensor.matmul(out_ap,
                             ltt[:, em['ltcol']:em['ltcol'] + em['width']],
                             buf[:, v:v + W],
                             start=start, stop=(mi == nmm - 1))
            mi += 1
        if j == 0 and p == 0:
            sbase = (blkinfo['espc'] % RING) * SLOTW
            for jj in range(NSPEC):
                zt = zp.tile([128, W], f16, tag="spz")
                nc.vector.tensor_mul(
                    zt, buf[:, sbase + 255 + jj:sbase + 255 + jj + W],
                    coeft[:, jj * W:(jj + 1) * W])
                nc.tensor.matmul(ps[64:128], ltst[:, jj * O:(jj + 1) * O], zt,
                                 start=False, stop=(mi == nmm - 1))
                mi += 1
        ot = outp.tile([128, W], f32, tag="out")
        nc.scalar.activation(ot, ps,
                             mybir.ActivationFunctionType.Identity,
                             bias=biast, scale=1.0)
        nc.sync.dma_start(outd[p], ot)

    for p in range(NPAIR):
        while staged < tgt[p]:
            stage(staged)
            staged += 1
        ltw = max(pairs[p]['ltw'], 64)
        ltt = ltp.tile([128, aps['LTW']], f16, tag="ltt")
        nc.sync.dma_start(ltt[:, :ltw], lt[p][:, :ltw])
        ltts[p] = ltt
        if p >= LTLEAD:
            emit_pair(p - LTLEAD)
    for p in range(NPAIR - LTLEAD, NPAIR):
        emit_pair(p)


tiles_pools = [None]


def _emit_kernel(tc, aps, blocks, E):
    import concourse.mybir as mybir
    nc = tc.nc
    f16 = mybir.dt.float16
    f32 = mybir.dt.float32

    with tc.tile_pool(name="bigp", bufs=1) as bigp, \
         tc.tile_pool(name="ltp", bufs=LTLEAD + 2) as ltp, \
         tc.tile_pool(name="zp", bufs=3) as zp, \
         tc.tile_pool(name="psp", bufs=6, space="PSUM") as psp, \
         tc.tile_pool(name="outp", bufs=3) as outp:

        buf = bigp.tile([128, RING * SLOTW], f16)
        coeft = bigp.tile([128, NSPEC * W], f16)
        biast = bigp.tile([128, 1], f32)
        ltst = bigp.tile([128, NSPEC * O], f16)

        blkv = nc.values_load(aps['blkid'][0:1, 0:1],
                              min_val=0, max_val=3,
                              skip_runtime_bounds_check=True)

        nc.sync.dma_start(coeft, aps['coefr'])
        nc.sync.dma_start(biast, aps['biasd'])
        nc.sync.dma_start(ltst, aps['lts'])

        # hoisted band-agnostic prologue staging of the first RING events
        # (overlaps the blkid load + branch resolution)
        for e in range(min(RING, E)):
            base = (e % RING) * SLOTW
            src = aps['xb'][e].rearrange("p c w -> (p c) w")
            nc.sync.dma_start(buf[:, base:base + W], src)
            nc.sync.dma_start(buf[:, base + W:base + 2 * W], src)
            nc.vector.tensor_copy(buf[:, base + GOFF:base + GOFF + W],
                                  buf[:, base + 1:base + 1 + W])
            nc.scalar.copy(buf[:, base + GOFF + W:base + GOFF + 2 * W],
                           buf[:, base + 1:base + 1 + W])
            nc.gpsimd.memset(buf[:, base + GOFF + 511:base + GOFF + 512], 0.0)

        tiles = (buf, coeft, biast, ltst)
        tiles_pools[0] = (psp, ltp, zp, outp)
        for j in range(4):
            with tc.If(blkv == j):
                _emit_pair_section(tc, aps, tiles, blocks[j], j)


def _get_compiled():
    """Build tables, schedule, and the Bass program once."""
    if 'prog' in _CACHE:
        return _CACHE['prog']
    import concourse.mybir as mybir
    import concourse.tile as tile
    from concourse import bacc

    tt = _build_tap_tables()
    blocks, E, LTW = _build_schedule(tt)
    # special-slot event index (input row of tap (1,1)) for band 0
    espc = None
    iy_spc = int(np.clip(tt['iy0'][1, 1], 0, 255))
    for ei, iy in enumerate(blocks[0]['events']):
        if iy == iy_spc:
            espc = ei
            break
    assert espc is not None
    blocks[0]['espc'] = espc
    for j in range(1, 4):
        blocks[j]['espc'] = 0

    f16 = mybir.dt.float16
    f32 = mybir.dt.float32
    nc = bacc.Bacc("TRN2", target_bir_lowering=False, debug=False,
                   num_devices=NCORES)
    aps = {
        'xb': nc.dram_tensor("xb", [E, 2, C, W], f16,
                             kind="ExternalInput").ap(),
        'lt': nc.dram_tensor("lt", [NPAIR, 128, LTW], f16,
                             kind="ExternalInput").ap(),
        'lts': nc.dram_tensor("lts", [128, NSPEC * O], f16,
                              kind="ExternalInput").ap(),
        'blkid': nc.dram_tensor("blkid", [1, 1], mybir.dt.int32,
                                kind="ExternalInput").ap(),
        'coefr': nc.dram_tensor("coefr", [128, NSPEC * W], f16,
                                kind="ExternalInput").ap(),
        'biasd': nc.dram_tensor("biasd", [128, 1], f32,
                                kind="ExternalInput").ap(),
        'out': nc.dram_tensor("out", [NPAIR, 128, W], f32,
                              kind="ExternalOutput").ap(),
        'LTW': LTW,
    }
    with tile.TileContext(nc) as tc:
        _emit_kernel(tc, aps, blocks, E)
    nc.finalize()

    _CACHE['prog'] = (nc, tt, blocks, E, LTW)
    return _CACHE['prog']


def _core_inputs(x, weight, bias, tt, blocks, E, LTW):
    """Assemble per-core in_maps. Core c = batch (c // 4), band (c % 4)."""
    w3 = weight.reshape(O, C, K).astype(np.float64)
    # wT[k]: [c, o] weight slice per tap
    wT = [np.ascontiguousarray(w3[:, :, k].T) for k in range(K)]
    biasd = np.ascontiguousarray(
        np.concatenate([bias, bias]).reshape(128, 1).astype(np.float32))

    lts_on = np.zeros((128, NSPEC * O), np.float16)
    for jj in range(NSPEC):
        lts_on[:C, jj * O:(jj + 1) * O] = wT[1].astype(np.float16)
    lts_off = np.zeros((128, NSPEC * O), np.float16)

    Gam = tt['Gam'].astype(np.float16)
    coef_on = np.ascontiguousarray(
        np.broadcast_to(Gam[:, None, :], (NSPEC, 128, W))
        .transpose(1, 0, 2).reshape(128, NSPEC * W))
    coef_off = np.zeros((128, NSPEC * W), np.float16)

    lt_blk = []
    for blk in range(4):
        ltv = np.zeros((NPAIR, 128, LTW), np.float64)
        for p in range(NPAIR):
            for em in blocks[blk]['pairs'][p]['emits']:
                c = em['ltcol']
                for k, (sc0, sc1) in em['top'].items():
                    ltv[p, 0:64, c:c + 64] += sc0 * wT[k]
                    ltv[p, 64:128, c:c + 64] += sc1 * wT[k]
                cb = c + 64 if em['merged'] else c
                for k, (sc0, sc1) in em['bot'].items():
                    ltv[p, 0:64, cb:cb + 64] += sc0 * wT[k]
                    ltv[p, 64:128, cb:cb + 64] += sc1 * wT[k]
        lt_blk.append(ltv.astype(np.float16))

    in_maps = []
    for cid in range(NCORES):
        b, blk = cid // 4, cid % 4
        xz = np.concatenate([x[b], np.zeros((C, 1, W), x.dtype)], axis=1)
        xz = xz.astype(np.float16)
        rows = np.asarray(blocks[blk]['events'], np.int64)
        pair_idx = np.stack([rows, rows + 1], axis=1)       # [E, 2]
        xbv = xz[:, pair_idx, :]                            # [C, E, 2, W]
        xbv = np.ascontiguousarray(xbv.transpose(1, 2, 0, 3))  # [E,2,C,W]
        in_maps.append({
            'xb': xbv,
            'lt': lt_blk[blk],
            'lts': lts_on if blk == 0 else lts_off,
            'blkid': np.array([[blk]], np.int32),
            'coefr': coef_on if blk == 0 else coef_off,
            'biasd': biasd,
        })
    return in_maps


def _gather(res):
    """Assemble full output from per-core results."""
    out = np.empty((B, O, H, W), np.float32)
    for cid in range(NCORES):
        b, blk = cid // 4, cid % 4
        oc = res.results[cid]['out']                        # [NPAIR, 128, W]
        h0 = blk * NROW
        out[b, :, h0 + 0:h0 + NROW:2, :] = oc[:, 0:64, :].transpose(1, 0, 2)
        out[b, :, h0 + 1:h0 + NROW:2, :] = oc[:, 64:128, :].transpose(1, 0, 2)
    return out


def kernel(x, weight, bias):
    from concourse.bass_utils import run_bass_kernel_spmd
    x = np.asarray(x, dtype=np.float32)
    weight = np.asarray(weight, dtype=np.float32)
    bias = np.asarray(bias, dtype=np.float32)

    nc, tt, blocks, E, LTW = _get_compiled()
    in_maps = _core_inputs(x, weight, bias, tt, blocks, E, LTW)
    res = run_bass_kernel_spmd(nc, in_maps, core_ids=list(range(NCORES)))
    return _gather(res)


# revision 14
# speedup vs baseline: 1.4658x; 1.0054x over previous
"""Trainium2 Bass kernel for nn_EquiConv2d (equirectangular deformable conv).

Key structural facts exploited (derived from the reference geometry):
  * off_y is exactly longitude-invariant, so each (tap k, row h) samples two
    fixed input rows (iy0, iy0+1) with a constant y-fraction.
  * off_x is longitude-invariant up to the 2*pi wrap: sampling along a row is
    a CIRCULAR shift by a constant s0(k,h) plus a constant x-fraction.
  * Hence the deformable conv is a set of matmul "slots" per output row
    ([128=(c x row-pair) contraction, 512 free]) reading circularly
    duplicated row-pair tiles at per-(k,h) column offsets, with the bilinear
    corner weights folded into the stationary (weight) operand.
  * NEW vs baseline: output rows are processed in PAIRS sharing one PSUM
    bank (top 64 partitions = even row, bottom 64 = odd row).  Slots of the
    two rows that read the SAME moving stream (same input row-pair event,
    same column window, same wrap-seam variant) are MERGED into a single
    matmul with a [128, 128] stationary — the ky-ladder of the equirect
    geometry makes ~30% of all slots mergeable, cutting tensor-engine work
    by the same fraction with bit-identical arithmetic.
  * Two fp32 oddities handled exactly: tap (k=7,h=255) is identically zero
    and tap (k=1,h=1) samples near the antipode with fp32-noise-scattered
    positions -> handled by 3 extra matmul slots with per-column coefficient
    vectors (data-driven, active only on the cores owning global row 1).

Sharding: 8 cores = 2 batches x 4 bands of 32 output-row pairs.
"""

import math

import numpy as np

# ----------------------------------------------------------------------------
# problem constants
B, C, H, W = 2, 64, 256, 512
O, KH, KW = 64, 3, 3
K = KH * KW
NCORES = 8
NROW = 64            # output rows per core
NPAIR = NROW // 2    # output row-pairs per core
NSPEC = 3            # special (antipode) slots, accumulated into local row 1
RING = 24            # staged row-pair ring slots
PF = 2               # staging prefetch lead (row-pairs)
LTLEAD = 2           # lt-table DMA prefetch lead (row-pairs)
SLOTW = 1536         # [A=row | B=row | Z=row w/ col0 zeroed] per ring slot
SKIP_TOL = 1e-4      # drop matmul slots with |scale| below this

_CACHE = {}


# ----------------------------------------------------------------------------
# host-side geometry tables (must replicate reference fp32 semantics exactly)

def _compute_offsets_jax():
    """Bit-exact replica of reference.equi_offsets on jax CPU."""
    import jax
    import jax.numpy as jnp
    cpu = jax.devices("cpu")[0]
    with jax.default_device(cpu):
        dtype = jnp.float32
        pano_H, pano_W, kH, kW = H, W, KH, KW
        Kk = kH * kW
        u = jnp.arange(pano_W, dtype=dtype)
        v = jnp.arange(pano_H, dtype=dtype)
        phi = (u - pano_W / 2.0) / pano_W * (2.0 * math.pi)
        theta = -(v - pano_H / 2.0) / pano_H * math.pi
        cp, sp = jnp.cos(phi), jnp.sin(phi)
        z, one = jnp.zeros_like(cp), jnp.ones_like(cp)
        Ry = jnp.stack([jnp.stack([cp, z, sp], -1),
                        jnp.stack([z, one, z], -1),
                        jnp.stack([-sp, z, cp], -1)], -2)
        ct, st = jnp.cos(theta), jnp.sin(theta)
        zh, oh = jnp.zeros_like(ct), jnp.ones_like(ct)
        Rx = jnp.stack([jnp.stack([oh, zh, zh], -1),
                        jnp.stack([zh, ct, -st], -1),
                        jnp.stack([zh, st, ct], -1)], -2)
        ROT = jnp.einsum('wij,hjk->hwik', Ry, Rx)
        fov_w = kW * (2.0 * math.pi / pano_W)
        focal = (kW / 2.0) / math.tan(fov_w / 2.0)
        hg = (jnp.arange(kH, dtype=dtype)[:, None] + 0.5 - kH / 2.0)
        wg = (jnp.arange(kW, dtype=dtype)[None, :] + 0.5 - kW / 2.0)
        hg = jnp.broadcast_to(hg, (kH, kW)).reshape(Kk)
        wg = jnp.broadcast_to(wg, (kH, kW)).reshape(Kk)
        rays0 = jnp.stack([wg / focal, hg / focal, jnp.ones(Kk, dtype)], 0)
        rays0 = rays0 / jnp.linalg.norm(rays0, axis=0, keepdims=True)
        rays = jnp.einsum('hwik,kn->hwin', ROT, rays0)
        phi2 = jnp.arctan2(rays[..., 0, :], rays[..., 2, :])
        th2 = jnp.arcsin(jnp.clip(rays[..., 1, :], -1.0, 1.0))
        x = pano_W / (2.0 * math.pi) * phi2 + pano_W / 2.0
        y = pano_H / math.pi * th2 + pano_H / 2.0
        off_x = x - (wg[None, None, :] + u[None, :, None])
        off_y = y - (hg[None, None, :] + v[:, None, None])
        return (np.asarray(jnp.transpose(off_y, (2, 0, 1))),
                np.asarray(jnp.transpose(off_x, (2, 0, 1))))


def _build_tap_tables():
    off_y, off_x = _compute_offsets_jax()
    ky = np.repeat(np.arange(KH), KW).astype(np.float32)
    kx = np.tile(np.arange(KW), KH).astype(np.float32)
    base_x = (np.arange(W, dtype=np.float32) - np.float32(1))
    base_y = (np.arange(H, dtype=np.float32) - np.float32(1))
    px = (base_x[None, None, :] + kx[:, None, None] + off_x).astype(np.float32)
    py = (base_y[None, :, None] + ky[:, None, None] + off_y).astype(np.float32)
    pyc = py[:, :, 0]
    assert np.all(py == pyc[:, :, None]), "off_y not longitude-invariant"

    iy0 = np.floor(pyc).astype(np.int64)
    wy1 = (pyc - np.floor(pyc)).astype(np.float64)
    v0 = (iy0 >= 0) & (iy0 < H)
    v1 = (iy0 + 1 >= 0) & (iy0 + 1 < H)
    cy0 = np.where(v0, 1.0 - wy1, 0.0)
    cy1 = np.where(v1, wy1, 0.0)

    Draw = np.mod((px.astype(np.float64) - np.arange(W)[None, None, :]), 512.0)
    ang = Draw / 512.0 * 2 * np.pi
    mean = np.mod(np.angle(np.exp(1j * ang).mean(axis=2)) / (2 * np.pi) * 512.0,
                  512.0)
    resid = np.mod(Draw - mean[:, :, None] + 256.0, 512.0) - 256.0
    D = mean + np.median(resid, axis=2)
    s0 = np.mod(np.floor(D), 512).astype(np.int64)
    frac = D - np.floor(D)

    special = np.zeros((K, H), dtype=bool)
    special[1, 1] = True
    dead = (cy0 == 0.0) & (cy1 == 0.0)

    Ddev = np.abs(np.mod(Draw - D[:, :, None] + 256.0, 512.0) - 256.0)
    dev = Ddev.max(axis=2)
    bad = (dev > 5e-4) & ~special & ~dead
    assert not bad.any(), f"unrepresentable taps: {np.argwhere(bad)}"

    def ref_coefs(p):
        x0 = math.floor(p)
        fr = p - x0
        out = {}
        for ix, wt in ((x0, 1.0 - fr), (x0 + 1, fr)):
            if 0 <= ix < W and wt != 0.0:
                out[ix] = out.get(ix, 0.0) + wt
        return out

    # seam variant selection: decided by the exact fp32 px at the wrap column
    slot0_useG = np.zeros((K, H), dtype=bool)
    slot1_useF = np.zeros((K, H), dtype=bool)
    for k in range(K):
        for h in range(H):
            if special[k, h] or dead[k, h]:
                continue
            s = int(s0[k, h]); fr = frac[k, h]
            if s >= 1:
                w0 = (512 - s) % 512
                rc = ref_coefs(float(px[k, h, w0]))
                slot0_useG[k, h] = (abs(rc.get(0, 0.0))
                                    < abs(rc.get(0, 0.0) - (1 - fr)))
            w1 = (511 - s) % 512
            rc = ref_coefs(float(px[k, h, w1]))
            slot1_useF[k, h] = (abs(rc.get(0, 0.0) - fr)
                                < abs(rc.get(0, 0.0)))

    # special tap (1,1): per-column coefficients on F offsets 255..257
    pxs = px[1, 1, :].astype(np.float64)
    Gam = np.zeros((3, W), dtype=np.float64)
    for w in range(W):
        p = pxs[w]
        x0 = math.floor(p)
        fr = p - x0
        for ix, wt in ((x0, 1.0 - fr), (x0 + 1, fr)):
            if 0 <= ix < W and wt != 0.0:
                found = False
                for jj in range(3):
                    if (255 + jj + w) % 512 == ix % 512:
                        Gam[jj, w] += wt
                        found = True
                        break
                assert found, (w, p, ix)

    return dict(iy0=iy0, cy0=cy0, cy1=cy1, s0=s0, frac=frac,
                slot0_useG=slot0_useG, slot1_useF=slot1_useF,
                special=special, dead=dead, Gam=Gam)


# ----------------------------------------------------------------------------
# slot -> (event row, window, variant) keys + merged emit schedule

def _row_slots(tt, h):
    """Slots of output row h: list of (key, k, half_scales).
    key = (input_row, window_col, zeroed_variant);
    scales = (coef_row0, coef_row1) to fold into the stationary halves."""
    out = []
    for k in range(K):
        if tt['dead'][k, h] or tt['special'][k, h]:
            continue
        s = int(tt['s0'][k, h]); fr = float(tt['frac'][k, h])
        iy = int(np.clip(tt['iy0'][k, h], 0, 255))
        c0 = float(tt['cy0'][k, h]); c1 = float(tt['cy1'][k, h])
        cmax = max(abs(c0), abs(c1))
        if (1.0 - fr) * cmax >= SKIP_TOL:
            zer = bool(tt['slot0_useG'][k, h]) and s >= 1
            out.append(((iy, s, zer), k, (c0 * (1 - fr), c1 * (1 - fr))))
        if fr * cmax >= SKIP_TOL:
            zer = not bool(tt['slot1_useF'][k, h])
            out.append(((iy, s + 1, zer), k, (c0 * fr, c1 * fr)))
    return out


def _build_schedule(tt):
    """Per band: event list (input row-pairs, in first-use order over pairs),
    staging targets, and per-pair emit lists.

    emit = dict(ev, win, zer, top=[(k,c0,c1)...], bot=[...], ltcol, width)
    ordered merged-first (within groups by event index)."""
    iy_spc = int(np.clip(tt['iy0'][1, 1], 0, 255))
    blocks = []
    for blk in range(4):
        ev_of, events, first_use = {}, [], []
        pairs = []
        for p in range(NPAIR):
            h0 = blk * NROW + 2 * p
            h1 = h0 + 1
            if blk == 0 and p == 0 and iy_spc not in ev_of:
                ev_of[iy_spc] = len(events)
                events.append(iy_spc)
                first_use.append(0)
            keymap = {}
            for (key, k, sc) in _row_slots(tt, h0):
                keymap.setdefault(key, (dict(), dict()))[0].setdefault(
                    k, [0.0, 0.0])
                e = keymap[key][0][k]
                e[0] += sc[0]; e[1] += sc[1]
            for (key, k, sc) in _row_slots(tt, h1):
                keymap.setdefault(key, (dict(), dict()))[1].setdefault(
                    k, [0.0, 0.0])
                e = keymap[key][1][k]
                e[0] += sc[0]; e[1] += sc[1]
            # register events (input rows) in deterministic order
            emits = []
            for key in keymap:
                iy = key[0]
                if iy not in ev_of:
                    ev_of[iy] = len(events)
                    events.append(iy)
                    first_use.append(p)
                top, bot = keymap[key]
                emits.append(dict(ev=ev_of[iy], win=key[1], zer=key[2],
                                  top=top, bot=bot,
                                  merged=bool(top) and bool(bot)))
            # merged first, then solos; within each group by event order
            emits.sort(key=lambda em: (not em['merged'], em['ev'], em['win']))
            # assign lt columns
            col = 0
            for em in emits:
                em['width'] = 128 if em['merged'] else 64
                em['ltcol'] = col
                col += em['width']
            pairs.append(dict(emits=emits, ltw=col))
        blocks.append(dict(events=events, first_use=first_use, pairs=pairs))

    E = max(len(b['events']) for b in blocks)
    LTW = max(pr['ltw'] for b in blocks for pr in b['pairs'])
    for b in blocks:
        while len(b['events']) < E:
            b['events'].append(b['events'][-1])

    # staging target per pair: staged-count needed before pair p is
    # tgt[p] = U[min(p+PF, NPAIR-1)] where U = events first-used by <= p
    for b in blocks:
        fu = np.asarray(b['first_use'])
        Uv = np.array([int(np.searchsorted(fu, p, 'right'))
                       for p in range(NPAIR)])
        b['tgt'] = [int(Uv[min(p + PF, NPAIR - 1)]) for p in range(NPAIR)]
        # ring-overwrite feasibility: stage(e) is issued in iteration ls[e]
        # AFTER emitting pair ls[e]-LTLEAD, so every reader of the slot it
        # overwrites (event e-RING) must have been emitted by then.  Matmul
        # emission lags staging by LTLEAD pairs, hence the margin.
        ls = np.full(E, NPAIR, np.int64)
        tgt = np.asarray(b['tgt'])
        for e in range(E):
            hit = np.where(tgt > e)[0]
            if len(hit):
                ls[e] = hit[0]
        lastuse = {}
        for p in range(NPAIR):
            for em in b['pairs'][p]['emits']:
                lastuse[em['ev']] = p
        for e in range(RING, E):
            if e - RING in lastuse:
                assert lastuse[e - RING] <= ls[e] - LTLEAD, \
                    f"RING={RING} too small: ev{e} overwrites ev{e-RING} " \
                    f"(lastuse pair {lastuse[e-RING]}, staged in it " \
                    f"{ls[e]}, emit lag {LTLEAD})"
    return blocks, E, LTW


# ----------------------------------------------------------------------------
# device program

def _emit_pair_section(tc, aps, tiles, blkinfo, j, ltts0):
    """Emit one per-band section (all-static APs)."""
    import concourse.mybir as mybir
    nc = tc.nc
    f16 = mybir.dt.float16
    f32 = mybir.dt.float32
    buf, coeft, biast, ltst = tiles
    xb, outd, lt = aps['xb'], aps['out'], aps['lt']
    psp, ltp, zp, outp = tiles_pools[0]
    tgt = blkinfo['tgt']
    pairs = blkinfo['pairs']

    def stage(e):
        base = (e % RING) * SLOTW
        src = xb[e].rearrange("p c w -> (p c) w")
        nc.sync.dma_start(buf[:, base:base + W], src)
        nc.vector.tensor_copy(buf[:, base + W:base + 2 * W],
                              buf[:, base:base + W])
        nc.vector.tensor_copy(buf[:, base + 2 * W + 1:base + 3 * W],
                              buf[:, base + 1:base + W])
        nc.gpsimd.memset(buf[:, base + 2 * W:base + 2 * W + 1], 0.0)

    staged = min(RING, len(blkinfo['events']))   # hoisted prologue staging
    ltts = [None] * NPAIR
    for p in range(min(LTLEAD, NPAIR)):
        ltts[p] = ltts0[p]                       # hoisted prologue lt tiles

    def emit_pair(p):
        pr = pairs[p]
        ltt = ltts[p]
        ps = psp.tile([128, W], f32, tag="ps")
        emits = pr['emits']
        nmm = len(emits) + (NSPEC if (j == 0 and p == 0) else 0)
        mi = 0
        started_top = started_bot = False
        for em in emits:
            base = (em['ev'] % RING) * SLOTW
            v = base + em['win'] if not em['zer'] \
                else base + W + em['win']
            if em['merged']:
                assert not (started_top or started_bot) or \
                    (started_top and started_bot)
                start = not started_top
                started_top = started_bot = True
                out_ap = ps
            elif em['top']:
                start = not started_top
                started_top = True
                out_ap = ps[0:64]
            else:
                start = not started_bot
                started_bot = True
                out_ap = ps[64:128]
            nc.tensor.matmul(out_ap,
                             ltt[:, em['ltcol']:em['ltcol'] + em['width']],
                             buf[:, v:v + W],
                             start=start, stop=(mi == nmm - 1))
            mi += 1
        if j == 0 and p == 0:
            sbase = (blkinfo['espc'] % RING) * SLOTW
            for jj in range(NSPEC):
                zt = zp.tile([128, W], f16, tag="spz")
                nc.vector.tensor_mul(
                    zt, buf[:, sbase + 255 + jj:sbase + 255 + jj + W],
                    coeft[:, jj * W:(jj + 1) * W])
                nc.tensor.matmul(ps[64:128], ltst[:, jj * O:(jj + 1) * O], zt,
                                 start=False, stop=(mi == nmm - 1))
                mi += 1
        ot = outp.tile([128, W], f32, tag="out")
        nc.scalar.activation(ot, ps,
                             mybir.ActivationFunctionType.Identity,
                             bias=biast, scale=1.0)
        nc.sync.dma_start(outd[p], ot)

    for p in range(NPAIR):
        if p >= LTLEAD:
            ltw = max(pairs[p]['ltw'], 64)
            ltt = ltp.tile([128, aps['LTW']], f16, tag="ltt")
            nc.sync.dma_start(ltt[:, :ltw], lt[p][:, :ltw])
            ltts[p] = ltt
            emit_pair(p - LTLEAD)
        while staged < tgt[p]:
            stage(staged)
            staged += 1
    for p in range(NPAIR - LTLEAD, NPAIR):
        emit_pair(p)


tiles_pools = [None]


def _emit_kernel(tc, aps, blocks, E):
    import concourse.mybir as mybir
    nc = tc.nc
    f16 = mybir.dt.float16
    f32 = mybir.dt.float32

    with tc.tile_pool(name="bigp", bufs=1) as bigp, \
         tc.tile_pool(name="ltp", bufs=LTLEAD + 2) as ltp, \
         tc.tile_pool(name="zp", bufs=3) as zp, \
         tc.tile_pool(name="psp", bufs=6, space="PSUM") as psp, \
         tc.tile_pool(name="outp", bufs=3) as outp:

        buf = bigp.tile([128, RING * SLOTW], f16)
        coeft = bigp.tile([128, NSPEC * W], f16)
        biast = bigp.tile([128, 1], f32)
        ltst = bigp.tile([128, NSPEC * O], f16)

        blkv = nc.values_load(aps['blkid'][0:1, 0:1],
                              min_val=0, max_val=3,
                              skip_runtime_bounds_check=True)

        # hoisted band-agnostic lt prefetch for the first LTLEAD pairs
        ltts0 = []
        for p in range(LTLEAD):
            ltt = ltp.tile([128, aps['LTW']], f16, tag="ltt")
            nc.sync.dma_start(ltt, aps['lt'][p])
            ltts0.append(ltt)
        nc.scalar.dma_start(coeft, aps['coefr'])
        nc.scalar.dma_start(biast, aps['biasd'])
        nc.scalar.dma_start(ltst, aps['lts'])

        # hoisted band-agnostic prologue staging of the first RING events
        # (overlaps the blkid load + dispatch), issue-split across queues
        for e in range(min(RING, E)):
            base = (e % RING) * SLOTW
            src = aps['xb'][e].rearrange("p c w -> (p c) w")
            q = nc.sync if e % 2 == 0 else nc.scalar
            q.dma_start(buf[:, base:base + W], src)
            nc.vector.tensor_copy(buf[:, base + W:base + 2 * W],
                                  buf[:, base:base + W])
            nc.vector.tensor_copy(buf[:, base + 2 * W + 1:base + 3 * W],
                                  buf[:, base + 1:base + W])
            nc.gpsimd.memset(buf[:, base + 2 * W:base + 2 * W + 1], 0.0)

        tiles = (buf, coeft, biast, ltst)
        tiles_pools[0] = (psp, ltp, zp, outp)
        for j in tc.Switch(blkv, 4):
            _emit_pair_section(tc, aps, tiles, blocks[j], j, ltts0)


def _get_compiled():
    """Build tables, schedule, and the Bass program once."""
    if 'prog' in _CACHE:
        return _CACHE['prog']
    import concourse.mybir as mybir
    import concourse.tile as tile
    from concourse import bacc

    tt = _build_tap_tables()
    blocks, E, LTW = _build_schedule(tt)
    # special-slot event index (input row of tap (1,1)) for band 0
    espc = None
    iy_spc = int(np.clip(tt['iy0'][1, 1], 0, 255))
    for ei, iy in enumerate(blocks[0]['events']):
        if iy == iy_spc:
            espc = ei
            break
    assert espc is not None
    blocks[0]['espc'] = espc
    for j in range(1, 4):
        blocks[j]['espc'] = 0

    f16 = mybir.dt.float16
    f32 = mybir.dt.float32
    nc = bacc.Bacc("TRN2", target_bir_lowering=False, debug=False,
                   num_devices=NCORES)
    aps = {
        'xb': nc.dram_tensor("xb", [E, 2, C, W], f16,
                             kind="ExternalInput").ap(),
        'lt': nc.dram_tensor("lt", [NPAIR, 128, LTW], f16,
                             kind="ExternalInput").ap(),
        'lts': nc.dram_tensor("lts", [128, NSPEC * O], f16,
                              kind="ExternalInput").ap(),
        'blkid': nc.dram_tensor("blkid", [1, 1], mybir.dt.int32,
                                kind="ExternalInput").ap(),
        'coefr': nc.dram_tensor("coefr", [128, NSPEC * W], f16,
                                kind="ExternalInput").ap(),
        'biasd': nc.dram_tensor("biasd", [128, 1], f32,
                                kind="ExternalInput").ap(),
        'out': nc.dram_tensor("out", [NPAIR, 128, W], f32,
                              kind="ExternalOutput").ap(),
        'LTW': LTW,
    }
    with tile.TileContext(nc) as tc:
        _emit_kernel(tc, aps, blocks, E)
    nc.finalize()

    _CACHE['prog'] = (nc, tt, blocks, E, LTW)
    return _CACHE['prog']


def _core_inputs(x, weight, bias, tt, blocks, E, LTW):
    """Assemble per-core in_maps. Core c = batch (c // 4), band (c % 4)."""
    w3 = weight.reshape(O, C, K).astype(np.float64)
    # wT[k]: [c, o] weight slice per tap
    wT = [np.ascontiguousarray(w3[:, :, k].T) for k in range(K)]
    biasd = np.ascontiguousarray(
        np.concatenate([bias, bias]).reshape(128, 1).astype(np.float32))

    lts_on = np.zeros((128, NSPEC * O), np.float16)
    for jj in range(NSPEC):
        lts_on[:C, jj * O:(jj + 1) * O] = wT[1].astype(np.float16)
    lts_off = np.zeros((128, NSPEC * O), np.float16)

    Gam = tt['Gam'].astype(np.float16)
    coef_on = np.ascontiguousarray(
        np.broadcast_to(Gam[:, None, :], (NSPEC, 128, W))
        .transpose(1, 0, 2).reshape(128, NSPEC * W))
    coef_off = np.zeros((128, NSPEC * W), np.float16)

    lt_blk = []
    for blk in range(4):
        ltv = np.zeros((NPAIR, 128, LTW), np.float64)
        for p in range(NPAIR):
            for em in blocks[blk]['pairs'][p]['emits']:
                c = em['ltcol']
                for k, (sc0, sc1) in em['top'].items():
                    ltv[p, 0:64, c:c + 64] += sc0 * wT[k]
                    ltv[p, 64:128, c:c + 64] += sc1 * wT[k]
                cb = c + 64 if em['merged'] else c
                for k, (sc0, sc1) in em['bot'].items():
                    ltv[p, 0:64, cb:cb + 64] += sc0 * wT[k]
                    ltv[p, 64:128, cb:cb + 64] += sc1 * wT[k]
        lt_blk.append(ltv.astype(np.float16))

    in_maps = []
    for cid in range(NCORES):
        b, blk = cid // 4, cid % 4
        xz = np.concatenate([x[b], np.zeros((C, 1, W), x.dtype)], axis=1)
        xz = xz.astype(np.float16)
        rows = np.asarray(blocks[blk]['events'], np.int64)
        pair_idx = np.stack([rows, rows + 1], axis=1)       # [E, 2]
        xbv = xz[:, pair_idx, :]                            # [C, E, 2, W]
        xbv = np.ascontiguousarray(xbv.transpose(1, 2, 0, 3))  # [E,2,C,W]
        in_maps.append({
            'xb': xbv,
            'lt': lt_blk[blk],
            'lts': lts_on if blk == 0 else lts_off,
            'blkid': np.array([[blk]], np.int32),
            'coefr': coef_on if blk == 0 else coef_off,
            'biasd': biasd,
        })
    return in_maps


def _gather(res):
    """Assemble full output from per-core results."""
    out = np.empty((B, O, H, W), np.float32)
    for cid in range(NCORES):
        b, blk = cid // 4, cid % 4
        oc = res.results[cid]['out']                        # [NPAIR, 128, W]
        h0 = blk * NROW
        out[b, :, h0 + 0:h0 + NROW:2, :] = oc[:, 0:64, :].transpose(1, 0, 2)
        out[b, :, h0 + 1:h0 + NROW:2, :] = oc[:, 64:128, :].transpose(1, 0, 2)
    return out


def kernel(x, weight, bias):
    from concourse.bass_utils import run_bass_kernel_spmd
    x = np.asarray(x, dtype=np.float32)
    weight = np.asarray(weight, dtype=np.float32)
    bias = np.asarray(bias, dtype=np.float32)

    nc, tt, blocks, E, LTW = _get_compiled()
    in_maps = _core_inputs(x, weight, bias, tt, blocks, E, LTW)
    res = run_bass_kernel_spmd(nc, in_maps, core_ids=list(range(NCORES)))
    return _gather(res)


# revision 19
# speedup vs baseline: 1.5870x; 1.0827x over previous
"""Trainium2 Bass kernel for nn_EquiConv2d (equirectangular deformable conv).

Key structural facts exploited (derived from the reference geometry):
  * off_y is exactly longitude-invariant, so each (tap k, row h) samples two
    fixed input rows (iy0, iy0+1) with a constant y-fraction.
  * off_x is longitude-invariant up to the 2*pi wrap: sampling along a row is
    a CIRCULAR shift by a constant s0(k,h) plus a constant x-fraction.
  * Hence the deformable conv is a set of matmul "slots" per output row
    ([128=(c x row-pair) contraction, 512 free]) reading circularly
    duplicated row-pair tiles at per-(k,h) column offsets, with the bilinear
    corner weights folded into the stationary (weight) operand.
  * NEW vs baseline: output rows are processed in PAIRS sharing one PSUM
    bank (top 64 partitions = even row, bottom 64 = odd row).  Slots of the
    two rows that read the SAME moving stream (same input row-pair event,
    same column window, same wrap-seam variant) are MERGED into a single
    matmul with a [128, 128] stationary — the ky-ladder of the equirect
    geometry makes ~30% of all slots mergeable, cutting tensor-engine work
    by the same fraction with bit-identical arithmetic.
  * Two fp32 oddities handled exactly: tap (k=7,h=255) is identically zero
    and tap (k=1,h=1) samples near the antipode with fp32-noise-scattered
    positions -> handled by 3 extra matmul slots with per-column coefficient
    vectors (data-driven, active only on the cores owning global row 1).

Sharding: 8 cores = 2 batches x 4 bands of 32 output-row pairs.
"""

import math

import numpy as np

# ----------------------------------------------------------------------------
# problem constants
B, C, H, W = 2, 64, 256, 512
O, KH, KW = 64, 3, 3
K = KH * KW
NCORES = 8
NROW = 64            # output rows per core
NPAIR = NROW // 2    # output row-pairs per core
NSPEC = 3            # special (antipode) slots, accumulated into local row 1
RING = 24            # staged row-pair ring slots
PF = 2               # staging prefetch lead (row-pairs)
LTLEAD = 3           # lt-table DMA prefetch lead (row-pairs)
SLOTW = 1536         # [A=row | B=row | Z=row w/ col0 zeroed] per ring slot
SKIP_TOL = 1e-4      # drop matmul slots with |scale| below this

_CACHE = {}


# ----------------------------------------------------------------------------
# host-side geometry tables (must replicate reference fp32 semantics exactly)

def _compute_offsets_jax():
    """Bit-exact replica of reference.equi_offsets on jax CPU."""
    import jax
    import jax.numpy as jnp
    cpu = jax.devices("cpu")[0]
    with jax.default_device(cpu):
        dtype = jnp.float32
        pano_H, pano_W, kH, kW = H, W, KH, KW
        Kk = kH * kW
        u = jnp.arange(pano_W, dtype=dtype)
        v = jnp.arange(pano_H, dtype=dtype)
        phi = (u - pano_W / 2.0) / pano_W * (2.0 * math.pi)
        theta = -(v - pano_H / 2.0) / pano_H * math.pi
        cp, sp = jnp.cos(phi), jnp.sin(phi)
        z, one = jnp.zeros_like(cp), jnp.ones_like(cp)
        Ry = jnp.stack([jnp.stack([cp, z, sp], -1),
                        jnp.stack([z, one, z], -1),
                        jnp.stack([-sp, z, cp], -1)], -2)
        ct, st = jnp.cos(theta), jnp.sin(theta)
        zh, oh = jnp.zeros_like(ct), jnp.ones_like(ct)
        Rx = jnp.stack([jnp.stack([oh, zh, zh], -1),
                        jnp.stack([zh, ct, -st], -1),
                        jnp.stack([zh, st, ct], -1)], -2)
        ROT = jnp.einsum('wij,hjk->hwik', Ry, Rx)
        fov_w = kW * (2.0 * math.pi / pano_W)
        focal = (kW / 2.0) / math.tan(fov_w / 2.0)
        hg = (jnp.arange(kH, dtype=dtype)[:, None] + 0.5 - kH / 2.0)
        wg = (jnp.arange(kW, dtype=dtype)[None, :] + 0.5 - kW / 2.0)
        hg = jnp.broadcast_to(hg, (kH, kW)).reshape(Kk)
        wg = jnp.broadcast_to(wg, (kH, kW)).reshape(Kk)
        rays0 = jnp.stack([wg / focal, hg / focal, jnp.ones(Kk, dtype)], 0)
        rays0 = rays0 / jnp.linalg.norm(rays0, axis=0, keepdims=True)
        rays = jnp.einsum('hwik,kn->hwin', ROT, rays0)
        phi2 = jnp.arctan2(rays[..., 0, :], rays[..., 2, :])
        th2 = jnp.arcsin(jnp.clip(rays[..., 1, :], -1.0, 1.0))
        x = pano_W / (2.0 * math.pi) * phi2 + pano_W / 2.0
        y = pano_H / math.pi * th2 + pano_H / 2.0
        off_x = x - (wg[None, None, :] + u[None, :, None])
        off_y = y - (hg[None, None, :] + v[:, None, None])
        return (np.asarray(jnp.transpose(off_y, (2, 0, 1))),
                np.asarray(jnp.transpose(off_x, (2, 0, 1))))


def _build_tap_tables():
    off_y, off_x = _compute_offsets_jax()
    ky = np.repeat(np.arange(KH), KW).astype(np.float32)
    kx = np.tile(np.arange(KW), KH).astype(np.float32)
    base_x = (np.arange(W, dtype=np.float32) - np.float32(1))
    base_y = (np.arange(H, dtype=np.float32) - np.float32(1))
    px = (base_x[None, None, :] + kx[:, None, None] + off_x).astype(np.float32)
    py = (base_y[None, :, None] + ky[:, None, None] + off_y).astype(np.float32)
    pyc = py[:, :, 0]
    assert np.all(py == pyc[:, :, None]), "off_y not longitude-invariant"

    iy0 = np.floor(pyc).astype(np.int64)
    wy1 = (pyc - np.floor(pyc)).astype(np.float64)
    v0 = (iy0 >= 0) & (iy0 < H)
    v1 = (iy0 + 1 >= 0) & (iy0 + 1 < H)
    cy0 = np.where(v0, 1.0 - wy1, 0.0)
    cy1 = np.where(v1, wy1, 0.0)

    Draw = np.mod((px.astype(np.float64) - np.arange(W)[None, None, :]), 512.0)
    ang = Draw / 512.0 * 2 * np.pi
    mean = np.mod(np.angle(np.exp(1j * ang).mean(axis=2)) / (2 * np.pi) * 512.0,
                  512.0)
    resid = np.mod(Draw - mean[:, :, None] + 256.0, 512.0) - 256.0
    D = mean + np.median(resid, axis=2)
    s0 = np.mod(np.floor(D), 512).astype(np.int64)
    frac = D - np.floor(D)

    special = np.zeros((K, H), dtype=bool)
    special[1, 1] = True
    dead = (cy0 == 0.0) & (cy1 == 0.0)

    Ddev = np.abs(np.mod(Draw - D[:, :, None] + 256.0, 512.0) - 256.0)
    dev = Ddev.max(axis=2)
    bad = (dev > 5e-4) & ~special & ~dead
    assert not bad.any(), f"unrepresentable taps: {np.argwhere(bad)}"

    def ref_coefs(p):
        x0 = math.floor(p)
        fr = p - x0
        out = {}
        for ix, wt in ((x0, 1.0 - fr), (x0 + 1, fr)):
            if 0 <= ix < W and wt != 0.0:
                out[ix] = out.get(ix, 0.0) + wt
        return out

    # seam variant selection: decided by the exact fp32 px at the wrap column
    slot0_useG = np.zeros((K, H), dtype=bool)
    slot1_useF = np.zeros((K, H), dtype=bool)
    for k in range(K):
        for h in range(H):
            if special[k, h] or dead[k, h]:
                continue
            s = int(s0[k, h]); fr = frac[k, h]
            if s >= 1:
                w0 = (512 - s) % 512
                rc = ref_coefs(float(px[k, h, w0]))
                slot0_useG[k, h] = (abs(rc.get(0, 0.0))
                                    < abs(rc.get(0, 0.0) - (1 - fr)))
            w1 = (511 - s) % 512
            rc = ref_coefs(float(px[k, h, w1]))
            slot1_useF[k, h] = (abs(rc.get(0, 0.0) - fr)
                                < abs(rc.get(0, 0.0)))

    # special tap (1,1): per-column coefficients on F offsets 255..257
    pxs = px[1, 1, :].astype(np.float64)
    Gam = np.zeros((3, W), dtype=np.float64)
    for w in range(W):
        p = pxs[w]
        x0 = math.floor(p)
        fr = p - x0
        for ix, wt in ((x0, 1.0 - fr), (x0 + 1, fr)):
            if 0 <= ix < W and wt != 0.0:
                found = False
                for jj in range(3):
                    if (255 + jj + w) % 512 == ix % 512:
                        Gam[jj, w] += wt
                        found = True
                        break
                assert found, (w, p, ix)

    return dict(iy0=iy0, cy0=cy0, cy1=cy1, s0=s0, frac=frac,
                slot0_useG=slot0_useG, slot1_useF=slot1_useF,
                special=special, dead=dead, Gam=Gam)


# ----------------------------------------------------------------------------
# slot -> (event row, window, variant) keys + merged emit schedule

def _row_slots(tt, h):
    """Slots of output row h: list of (key, k, half_scales).
    key = (input_row, window_col, zeroed_variant);
    scales = (coef_row0, coef_row1) to fold into the stationary halves."""
    out = []
    for k in range(K):
        if tt['dead'][k, h] or tt['special'][k, h]:
            continue
        s = int(tt['s0'][k, h]); fr = float(tt['frac'][k, h])
        iy = int(np.clip(tt['iy0'][k, h], 0, 255))
        c0 = float(tt['cy0'][k, h]); c1 = float(tt['cy1'][k, h])
        cmax = max(abs(c0), abs(c1))
        if (1.0 - fr) * cmax >= SKIP_TOL:
            zer = bool(tt['slot0_useG'][k, h]) and s >= 1
            out.append(((iy, s, zer), k, (c0 * (1 - fr), c1 * (1 - fr))))
        if fr * cmax >= SKIP_TOL:
            zer = not bool(tt['slot1_useF'][k, h])
            out.append(((iy, s + 1, zer), k, (c0 * fr, c1 * fr)))
    return out


def _build_schedule(tt):
    """Per band: event list (input row-pairs, in first-use order over pairs),
    staging targets, and per-pair emit lists.

    emit = dict(ev, win, zer, top=[(k,c0,c1)...], bot=[...], ltcol, width)
    ordered merged-first (within groups by event index)."""
    iy_spc = int(np.clip(tt['iy0'][1, 1], 0, 255))
    blocks = []
    for blk in range(4):
        ev_of, events, first_use = {}, [], []
        pairs = []
        for p in range(NPAIR):
            h0 = blk * NROW + 2 * p
            h1 = h0 + 1
            if blk == 0 and p == 0 and iy_spc not in ev_of:
                ev_of[iy_spc] = len(events)
                events.append(iy_spc)
                first_use.append(0)
            keymap = {}
            for (key, k, sc) in _row_slots(tt, h0):
                keymap.setdefault(key, (dict(), dict()))[0].setdefault(
                    k, [0.0, 0.0])
                e = keymap[key][0][k]
                e[0] += sc[0]; e[1] += sc[1]
            for (key, k, sc) in _row_slots(tt, h1):
                keymap.setdefault(key, (dict(), dict()))[1].setdefault(
                    k, [0.0, 0.0])
                e = keymap[key][1][k]
                e[0] += sc[0]; e[1] += sc[1]
            # register events (input rows) in deterministic order
            emits = []
            for key in keymap:
                iy = key[0]
                if iy not in ev_of:
                    ev_of[iy] = len(events)
                    events.append(iy)
                    first_use.append(p)
                top, bot = keymap[key]
                emits.append(dict(ev=ev_of[iy], win=key[1], zer=key[2],
                                  top=top, bot=bot,
                                  merged=bool(top) and bool(bot)))
            # merged first, then solos; within each group by event order
            emits.sort(key=lambda em: (not em['merged'], em['ev'], em['win']))
            # assign lt columns
            col = 0
            for em in emits:
                em['width'] = 128 if em['merged'] else 64
                em['ltcol'] = col
                col += em['width']
            pairs.append(dict(emits=emits, ltw=col))
        blocks.append(dict(events=events, first_use=first_use, pairs=pairs))

    E = max(len(b['events']) for b in blocks)
    LTW = max(pr['ltw'] for b in blocks for pr in b['pairs'])
    for b in blocks:
        while len(b['events']) < E:
            b['events'].append(b['events'][-1])

    # staging target per pair: staged-count needed before pair p is
    # tgt[p] = U[min(p+PF, NPAIR-1)] where U = events first-used by <= p
    for b in blocks:
        fu = np.asarray(b['first_use'])
        Uv = np.array([int(np.searchsorted(fu, p, 'right'))
                       for p in range(NPAIR)])
        b['tgt'] = [int(Uv[min(p + PF, NPAIR - 1)]) for p in range(NPAIR)]
        # ring-overwrite feasibility: stage(e) is issued in iteration ls[e]
        # AFTER emitting pair ls[e]-LTLEAD, so every reader of the slot it
        # overwrites (event e-RING) must have been emitted by then.  Matmul
        # emission lags staging by LTLEAD pairs, hence the margin.
        ls = np.full(E, NPAIR, np.int64)
        tgt = np.asarray(b['tgt'])
        for e in range(E):
            hit = np.where(tgt > e)[0]
            if len(hit):
                ls[e] = hit[0]
        lastuse = {}
        for p in range(NPAIR):
            for em in b['pairs'][p]['emits']:
                lastuse[em['ev']] = p
        for e in range(RING, E):
            if e - RING in lastuse:
                assert lastuse[e - RING] <= ls[e] - LTLEAD, \
                    f"RING={RING} too small: ev{e} overwrites ev{e-RING} " \
                    f"(lastuse pair {lastuse[e-RING]}, staged in it " \
                    f"{ls[e]}, emit lag {LTLEAD})"
    return blocks, E, LTW


# ----------------------------------------------------------------------------
# device program

def _emit_pair_section(tc, aps, tiles, blkinfo, j, ltts0):
    """Emit one per-band section (all-static APs)."""
    import concourse.mybir as mybir
    nc = tc.nc
    f16 = mybir.dt.float16
    f32 = mybir.dt.float32
    buf, coeft, biast, ltst = tiles
    xb, outd, lt = aps['xb'], aps['out'], aps['lt']
    psp, ltp, zp, outp = tiles_pools[0]
    tgt = blkinfo['tgt']
    pairs = blkinfo['pairs']

    def stage(e):
        base = (e % RING) * SLOTW
        src = xb[e].rearrange("p c w -> (p c) w")
        nc.sync.dma_start(buf[:, base:base + W], src)
        nc.vector.tensor_copy(buf[:, base + W:base + 2 * W],
                              buf[:, base:base + W])
        nc.vector.tensor_copy(buf[:, base + 2 * W + 1:base + 3 * W],
                              buf[:, base + 1:base + W])
        nc.gpsimd.memset(buf[:, base + 2 * W:base + 2 * W + 1], 0.0)

    staged = min(RING, len(blkinfo['events']))   # hoisted prologue staging
    ltts = [None] * NPAIR
    for p in range(min(LTLEAD, NPAIR)):
        ltts[p] = ltts0[p]                       # hoisted prologue lt tiles

    def emit_pair(p):
        pr = pairs[p]
        ltt = ltts[p]
        ps = psp.tile([128, W], f32, tag="ps")
        emits = pr['emits']
        nmm = len(emits) + (NSPEC if (j == 0 and p == 0) else 0)
        mi = 0
        started_top = started_bot = False
        for em in emits:
            base = (em['ev'] % RING) * SLOTW
            v = base + em['win'] if not em['zer'] \
                else base + W + em['win']
            if em['merged']:
                assert not (started_top or started_bot) or \
                    (started_top and started_bot)
                start = not started_top
                started_top = started_bot = True
                out_ap = ps
            elif em['top']:
                start = not started_top
                started_top = True
                out_ap = ps[0:64]
            else:
                start = not started_bot
                started_bot = True
                out_ap = ps[64:128]
            nc.tensor.matmul(out_ap,
                             ltt[:, em['ltcol']:em['ltcol'] + em['width']],
                             buf[:, v:v + W],
                             start=start, stop=(mi == nmm - 1))
            mi += 1
        if j == 0 and p == 0:
            sbase = (blkinfo['espc'] % RING) * SLOTW
            for jj in range(NSPEC):
                zt = zp.tile([128, W], f16, tag="spz")
                nc.vector.tensor_mul(
                    zt, buf[:, sbase + 255 + jj:sbase + 255 + jj + W],
                    coeft[:, jj * W:(jj + 1) * W])
                nc.tensor.matmul(ps[64:128], ltst[:, jj * O:(jj + 1) * O], zt,
                                 start=False, stop=(mi == nmm - 1))
                mi += 1
        ot = outp.tile([128, W], f32, tag="out")
        nc.scalar.activation(ot, ps,
                             mybir.ActivationFunctionType.Identity,
                             bias=biast, scale=1.0)
        # 4-way split across issue queues: four parallel DMA engines cut the
        # per-pair output latency (matters for the drain tail)
        hw2 = W // 2
        nc.sync.dma_start(outd[p][0:64, 0:hw2], ot[0:64, 0:hw2])
        nc.scalar.dma_start(outd[p][0:64, hw2:W], ot[0:64, hw2:W])
        nc.sync.dma_start(outd[p][64:128, 0:hw2], ot[64:128, 0:hw2])
        nc.scalar.dma_start(outd[p][64:128, hw2:W], ot[64:128, hw2:W])

    for p in range(NPAIR):
        if p >= LTLEAD:
            ltw = max(pairs[p]['ltw'], 64)
            hw2 = ltw // 2
            ltt = ltp.tile([128, aps['LTW']], f16, tag="ltt")
            nc.sync.dma_start(ltt[:, :hw2], lt[p][:, :hw2])
            nc.scalar.dma_start(ltt[:, hw2:ltw], lt[p][:, hw2:ltw])
            ltts[p] = ltt
            emit_pair(p - LTLEAD)
        while staged < tgt[p]:
            stage(staged)
            staged += 1
    for p in range(NPAIR - LTLEAD, NPAIR):
        emit_pair(p)


tiles_pools = [None]


def _emit_kernel(tc, aps, blocks, E):
    import concourse.mybir as mybir
    nc = tc.nc
    f16 = mybir.dt.float16
    f32 = mybir.dt.float32

    with tc.tile_pool(name="bigp", bufs=1) as bigp, \
         tc.tile_pool(name="ltp", bufs=LTLEAD + 2) as ltp, \
         tc.tile_pool(name="zp", bufs=3) as zp, \
         tc.tile_pool(name="psp", bufs=6, space="PSUM") as psp, \
         tc.tile_pool(name="outp", bufs=3) as outp:

        buf = bigp.tile([128, RING * SLOTW], f16)
        coeft = bigp.tile([128, NSPEC * W], f16)
        biast = bigp.tile([128, 1], f32)
        ltst = bigp.tile([128, NSPEC * O], f16)

        blkv = nc.values_load(aps['blkid'][0:1, 0:1],
                              min_val=0, max_val=3,
                              skip_runtime_bounds_check=True)

        # hoisted band-agnostic prologue: lt prefetch for the first LTLEAD
        # pairs interleaved with staging of the first RING events
        # (overlaps the blkid load + dispatch), issue-split across queues
        ltts0 = []
        for p in range(LTLEAD):
            ltt = ltp.tile([128, aps['LTW']], f16, tag="ltt")
            hw2 = aps['LTW'] // 2
            nc.sync.dma_start(ltt[:, :hw2], aps['lt'][p][:, :hw2])
            nc.scalar.dma_start(ltt[:, hw2:], aps['lt'][p][:, hw2:])
            ltts0.append(ltt)
            for e in range(4 * p, 4 * p + 4):
                if e >= min(RING, E):
                    continue
                base = (e % RING) * SLOTW
                src = aps['xb'][e].rearrange("p c w -> (p c) w")
                q = nc.sync if e % 2 == 0 else nc.scalar
                q.dma_start(buf[:, base:base + W], src)
                nc.vector.tensor_copy(buf[:, base + W:base + 2 * W],
                                      buf[:, base:base + W])
                nc.vector.tensor_copy(buf[:, base + 2 * W + 1:base + 3 * W],
                                      buf[:, base + 1:base + W])
                nc.gpsimd.memset(buf[:, base + 2 * W:base + 2 * W + 1], 0.0)
        nc.scalar.dma_start(coeft, aps['coefr'])
        nc.scalar.dma_start(biast, aps['biasd'])
        nc.scalar.dma_start(ltst, aps['lts'])
        for e in range(4 * LTLEAD, min(RING, E)):
            base = (e % RING) * SLOTW
            src = aps['xb'][e].rearrange("p c w -> (p c) w")
            q = nc.sync if e % 2 == 0 else nc.scalar
            q.dma_start(buf[:, base:base + W], src)
            nc.vector.tensor_copy(buf[:, base + W:base + 2 * W],
                                  buf[:, base:base + W])
            nc.vector.tensor_copy(buf[:, base + 2 * W + 1:base + 3 * W],
                                  buf[:, base + 1:base + W])
            nc.gpsimd.memset(buf[:, base + 2 * W:base + 2 * W + 1], 0.0)

        tiles = (buf, coeft, biast, ltst)
        tiles_pools[0] = (psp, ltp, zp, outp)
        for j in tc.Switch(blkv, 4):
            _emit_pair_section(tc, aps, tiles, blocks[j], j, ltts0)


def _get_compiled():
    """Build tables, schedule, and the Bass program once."""
    if 'prog' in _CACHE:
        return _CACHE['prog']
    import concourse.mybir as mybir
    import concourse.tile as tile
    from concourse import bacc

    tt = _build_tap_tables()
    blocks, E, LTW = _build_schedule(tt)
    # special-slot event index (input row of tap (1,1)) for band 0
    espc = None
    iy_spc = int(np.clip(tt['iy0'][1, 1], 0, 255))
    for ei, iy in enumerate(blocks[0]['events']):
        if iy == iy_spc:
            espc = ei
            break
    assert espc is not None
    blocks[0]['espc'] = espc
    for j in range(1, 4):
        blocks[j]['espc'] = 0

    f16 = mybir.dt.float16
    f32 = mybir.dt.float32
    nc = bacc.Bacc("TRN2", target_bir_lowering=False, debug=False,
                   num_devices=NCORES)
    aps = {
        'xb': nc.dram_tensor("xb", [E, 2, C, W], f16,
                             kind="ExternalInput").ap(),
        'lt': nc.dram_tensor("lt", [NPAIR, 128, LTW], f16,
                             kind="ExternalInput").ap(),
        'lts': nc.dram_tensor("lts", [128, NSPEC * O], f16,
                              kind="ExternalInput").ap(),
        'blkid': nc.dram_tensor("blkid", [1, 1], mybir.dt.int32,
                                kind="ExternalInput").ap(),
        'coefr': nc.dram_tensor("coefr", [128, NSPEC * W], f16,
                                kind="ExternalInput").ap(),
        'biasd': nc.dram_tensor("biasd", [128, 1], f32,
                                kind="ExternalInput").ap(),
        'out': nc.dram_tensor("out", [NPAIR, 128, W], f32,
                              kind="ExternalOutput").ap(),
        'LTW': LTW,
    }
    with tile.TileContext(nc) as tc:
        _emit_kernel(tc, aps, blocks, E)
    nc.finalize()

    _CACHE['prog'] = (nc, tt, blocks, E, LTW)
    return _CACHE['prog']


def _core_inputs(x, weight, bias, tt, blocks, E, LTW):
    """Assemble per-core in_maps. Core c = batch (c // 4), band (c % 4)."""
    w3 = weight.reshape(O, C, K).astype(np.float64)
    # wT[k]: [c, o] weight slice per tap
    wT = [np.ascontiguousarray(w3[:, :, k].T) for k in range(K)]
    biasd = np.ascontiguousarray(
        np.concatenate([bias, bias]).reshape(128, 1).astype(np.float32))

    lts_on = np.zeros((128, NSPEC * O), np.float16)
    for jj in range(NSPEC):
        lts_on[:C, jj * O:(jj + 1) * O] = wT[1].astype(np.float16)
    lts_off = np.zeros((128, NSPEC * O), np.float16)

    Gam = tt['Gam'].astype(np.float16)
    coef_on = np.ascontiguousarray(
        np.broadcast_to(Gam[:, None, :], (NSPEC, 128, W))
        .transpose(1, 0, 2).reshape(128, NSPEC * W))
    coef_off = np.zeros((128, NSPEC * W), np.float16)

    lt_blk = []
    for blk in range(4):
        ltv = np.zeros((NPAIR, 128, LTW), np.float64)
        for p in range(NPAIR):
            for em in blocks[blk]['pairs'][p]['emits']:
                c = em['ltcol']
                for k, (sc0, sc1) in em['top'].items():
                    ltv[p, 0:64, c:c + 64] += sc0 * wT[k]
                    ltv[p, 64:128, c:c + 64] += sc1 * wT[k]
                cb = c + 64 if em['merged'] else c
                for k, (sc0, sc1) in em['bot'].items():
                    ltv[p, 0:64, cb:cb + 64] += sc0 * wT[k]
                    ltv[p, 64:128, cb:cb + 64] += sc1 * wT[k]
        lt_blk.append(ltv.astype(np.float16))

    in_maps = []
    for cid in range(NCORES):
        b, blk = cid // 4, cid % 4
        xz = np.concatenate([x[b], np.zeros((C, 1, W), x.dtype)], axis=1)
        xz = xz.astype(np.float16)
        rows = np.asarray(blocks[blk]['events'], np.int64)
        pair_idx = np.stack([rows, rows + 1], axis=1)       # [E, 2]
        xbv = xz[:, pair_idx, :]                            # [C, E, 2, W]
        xbv = np.ascontiguousarray(xbv.transpose(1, 2, 0, 3))  # [E,2,C,W]
        in_maps.append({
            'xb': xbv,
            'lt': lt_blk[blk],
            'lts': lts_on if blk == 0 else lts_off,
            'blkid': np.array([[blk]], np.int32),
            'coefr': coef_on if blk == 0 else coef_off,
            'biasd': biasd,
        })
    return in_maps


def _gather(res):
    """Assemble full output from per-core results."""
    out = np.empty((B, O, H, W), np.float32)
    for cid in range(NCORES):
        b, blk = cid // 4, cid % 4
        oc = res.results[cid]['out']                        # [NPAIR, 128, W]
        h0 = blk * NROW
        out[b, :, h0 + 0:h0 + NROW:2, :] = oc[:, 0:64, :].transpose(1, 0, 2)
        out[b, :, h0 + 1:h0 + NROW:2, :] = oc[:, 64:128, :].transpose(1, 0, 2)
    return out


def kernel(x, weight, bias):
    from concourse.bass_utils import run_bass_kernel_spmd
    x = np.asarray(x, dtype=np.float32)
    weight = np.asarray(weight, dtype=np.float32)
    bias = np.asarray(bias, dtype=np.float32)

    nc, tt, blocks, E, LTW = _get_compiled()
    in_maps = _core_inputs(x, weight, bias, tt, blocks, E, LTW)
    res = run_bass_kernel_spmd(nc, in_maps, core_ids=list(range(NCORES)))
    return _gather(res)
